# revision 28
# baseline (speedup 1.0000x reference)
"""Causal multi-head attention (B=4, S=2048, D=1024, H=16, HD=64) on 8 NeuronCores.

Sharding: core c handles batch b=c//2 and head-group hg=c%2 (8 heads each).
Each core computes out^T_partial = Wo_hg^T @ ctx_hg^T for its (b, hg); the host
sums the two head-group partials per batch, transposes, and adds the bias.

v3 structure (fp8 DoubleRow generation + attention):
- Q/K projections run entirely in fp8e4m3 with DoubleRow matmuls (2 k-tiles
  per instruction, 2x PE throughput).  Score errors wash out in the softmax
  average, so fp8 q/k is accuracy-safe.
- The V projection and the attention A@V run in bf16 for key-tiles 0-3 /
  q-superblock 0: early causal rows average few keys, so fp8 noise there
  does not cancel and would blow the error budget.  Key-tiles 4-15 project
  V via fp8 DoubleRow; superblocks 1-3 run A@V as fp8 DoubleRow over
  key-tile PAIRS (one matmul per 256 keys), with exp writing fp8 directly.
- Per-head V slots are 80 wide (64 v + 1 ones + 15 pad) because the dual-fp8
  weight load requires the stationary free dim to be a multiple of 16; the
  ones column yields the softmax denominator for free at PSUM row 64.
- Diagonal-superblock masking: score matmuls extend over the masked/junk
  region (finite garbage), then e is multiplied by fp8 triangle/zero masks.
- Softmax denominators: reciprocal at partition 0 plus a K=1 broadcast
  matmul (no DMA for head A; head B normalizes at partitions 0-63 then DMAs).
- Output partials are written in bf16; the host sums in f32 and adds bias.
"""

import sys

for _p in ("/opt/trn_rl_repo",):
    if _p not in sys.path:
        sys.path.insert(0, _p)

import numpy as np
import ml_dtypes
from contextlib import ExitStack

import concourse.bacc as bacc
import concourse.tile as tile
from concourse import mybir
from concourse.bass_utils import run_bass_kernel_spmd

F32 = mybir.dt.float32
BF16 = mybir.dt.bfloat16
F8 = mybir.dt.float8e4
DR = mybir.MatmulPerfMode.DoubleRow
Exp = mybir.ActivationFunctionType.Exp
Mult = mybir.AluOpType.mult

B, S, D, H, HD = 4, 2048, 1024, 16, 64
NC = 8          # cores
HL = 8          # heads per core (head-group)
DH = HL * HD    # 512, per-core head dim
KT = D // 128   # 8 k-tiles over d_in
ST = S // 128   # 16 tiles over sequence
NB = S // 512   # 4 q-superblocks
NP = HL // 2    # 4 head pairs per core
VW = 80         # per-head V slot width: 64 v + 1 ones + 15 pad (16-aligned)
SCALE = 1.0 / np.sqrt(HD)


def _build_nc(debug=False):
    nc = bacc.Bacc("TRN2", target_bir_lowering=False)

    xT = nc.declare_dram_parameter("xT", [D, S], F8, isOutput=False)
    xTb = nc.declare_dram_parameter("xTb", [D, 512], BF16, isOutput=False)
    wq = nc.declare_dram_parameter("wq", [D, DH], F8, isOutput=False)
    wk = nc.declare_dram_parameter("wk", [D, DH], F8, isOutput=False)
    wv = nc.declare_dram_parameter("wv", [D, DH], F8, isOutput=False)
    wvb = nc.declare_dram_parameter("wvb", [D, DH], BF16, isOutput=False)
    wo = nc.declare_dram_parameter("wo", [DH, D], BF16, isOutput=False)
    tri = nc.declare_dram_parameter("tri", [128, 128], BF16, isOutput=False)
    outT = nc.declare_dram_parameter("outT", [D, S], BF16, isOutput=True)

    with tile.TileContext(nc) as tc, ExitStack() as ctx:
        const_pool = ctx.enter_context(tc.tile_pool(name="const", bufs=1))
        x_pool = ctx.enter_context(tc.tile_pool(name="x", bufs=1))
        w_pool = ctx.enter_context(tc.tile_pool(name="w", bufs=1))
        qk_pool = ctx.enter_context(tc.tile_pool(name="qk", bufs=1))
        v_pool = ctx.enter_context(tc.tile_pool(name="v", bufs=1))
        ctxT_pool = ctx.enter_context(tc.tile_pool(name="ctxT", bufs=1))
        e_pool = ctx.enter_context(tc.tile_pool(name="e", bufs=8))
        r_pool = ctx.enter_context(tc.tile_pool(name="r", bufs=1))
        o_pool = ctx.enter_context(tc.tile_pool(name="o", bufs=2))
        o3_pool = ctx.enter_context(tc.tile_pool(name="o3", bufs=1))
        ps_sp = ctx.enter_context(tc.tile_pool(name="ps_sp", bufs=2, space="PSUM"))
        ps_c = ctx.enter_context(tc.tile_pool(name="ps_c", bufs=1, space="PSUM"))
        ps_gen = ctx.enter_context(tc.tile_pool(name="ps_gen", bufs=2, space="PSUM"))

        # ---- constants ----
        trit = const_pool.tile([128, 128], BF16)
        nc.sync.dma_start(trit[:], tri[:])
        onesb = const_pool.tile([1, 64], BF16)
        nc.vector.memset(onesb[:], 1.0)
        # fp8 masks for the DoubleRow diag rounds: triangle, and [zeros|triangle]
        trit8 = const_pool.tile([128, 128], F8)
        nc.vector.tensor_copy(trit8[:], trit[:])
        zt8 = const_pool.tile([128, 256], F8)
        nc.vector.memset(zt8[:, 0:128], 0.0)
        nc.vector.tensor_copy(zt8[:, 128:256], trit[:])

        # ---- inputs ----
        # Per-k 2D slices (multi-dim dram APs under-synchronize on HW).
        # Priority order on one queue: a DMA's descriptors go out before the
        # next trigger's, so the front-critical tensors (bf16 wv + bf16 x
        # superblock 0 for emit_v(0..3), then q/k weights) finish first.
        wvt = w_pool.tile([128, KT, DH], F8, name="wvt")
        wvbt = w_pool.tile([128, KT, DH], BF16, name="wvbt")
        wqt = w_pool.tile([128, KT, DH], F8, name="wqt")
        wkt = w_pool.tile([128, KT, DH], F8, name="wkt")
        wot = w_pool.tile([128, DH // 128, D], BF16, name="wot")
        xs = [x_pool.tile([128, KT, 512], F8, name=f"xs{_}") for _ in range(NB)]
        xs0b = x_pool.tile([128, KT, 512], BF16, name="xs0b")

        def xsrc(s):
            return xT[:, 512 * s : 512 * (s + 1)].rearrange("(k p) c -> p k c", p=128)

        for k in range(KT):
            nc.sync.dma_start(wvbt[:, k, :], wvb[128 * k : 128 * (k + 1), :])
            nc.sync.dma_start(xs0b[:, k, :], xTb[128 * k : 128 * (k + 1), :])
        nc.sync.dma_start(wqt[:], wq.rearrange("(k p) c -> p k c", p=128)[:])
        nc.sync.dma_start(wkt[:], wk.rearrange("(k p) c -> p k c", p=128)[:])
        nc.sync.dma_start(xs[0][:], xsrc(0)[:])
        nc.sync.dma_start(xs[1][:], xsrc(1)[:])
        nc.sync.dma_start(xs[2][:], xsrc(2)[:])
        nc.sync.dma_start(wvt[:], wv.rearrange("(k p) c -> p k c", p=128)[:])
        nc.sync.dma_start(xs[3][:], xsrc(3)[:])
        nc.sync.dma_start(wot[:], wo.rearrange("(k p) c -> p k c", p=128)[:])

        # ---- persistent activations ----
        qTt = [qk_pool.tile([128, S], BF16, name=f"qT{_}") for _ in range(NP)]
        kTt = [qk_pool.tile([128, S], BF16, name=f"kT{_}") for _ in range(NP)]
        # bf16 V (65-wide slots) for key-tiles 0-3: superblock-0 A@V path
        vt = [v_pool.tile([128, HL * (HD + 1)], BF16, name=f"v{_}") for _ in range(4)]
        # fp8 V key-tile pairs (80-wide slots) for the DoubleRow A@V path
        vtp = [v_pool.tile([128, 2, HL * VW], F8, name=f"vp{_}") for _ in range(ST // 2)]
        ctxT = [ctxT_pool.tile([128, S], BF16, name=f"ctxT{_}") for _ in range(NP)]

        # ---- generation units ----
        # Each unit is one PSUM accumulation chain, split into two half-unit
        # closures for fine-grained interleaving with the attention rounds.
        def emit_v(st):
            """V for k-tile st.

            st 0-3 run in bf16 (both vt and vtp copies are written): early
            causal rows attend over few keys, so fp8 noise there does not
            average out.  st 4-15 use the fp8 DoubleRow path."""
            state = {}

            def half_a():
                nc.vector.memset(
                    vtp[st // 2].rearrange("p k (h c) -> p k h c", c=VW)[
                        :, st % 2, :, HD : VW
                    ],
                    1.0,
                )
                if st < 4:
                    nc.vector.memset(
                        vt[st].rearrange("p (h c) -> p h c", c=HD + 1)[:, :, HD], 1.0
                    )
                state["pv"] = ps_gen.tile([128, 512], F32, tag="pg", name=f"pv{st}")
                if st < 4:
                    for k in range(KT // 2):
                        nc.tensor.matmul(
                            state["pv"][:],
                            xs0b[:, k, 128 * st : 128 * (st + 1)],
                            wvbt[:, k, :],
                            start=(k == 0),
                            stop=False,
                        )
                else:
                    xcol = xs[st // 4]
                    for kp in range(KT // 4):
                        nc.tensor.matmul(
                            state["pv"][:],
                            xcol[:, 2 * kp : 2 * kp + 2, 128 * (st % 4) : 128 * (st % 4 + 1)],
                            wvt[:, 2 * kp : 2 * kp + 2, :],
                            start=(kp == 0),
                            stop=False,
                            perf_mode=DR,
                        )

            def half_b():
                pv = state["pv"]
                if st < 4:
                    for k in range(KT // 2, KT):
                        nc.tensor.matmul(
                            pv[:],
                            xs0b[:, k, 128 * st : 128 * (st + 1)],
                            wvbt[:, k, :],
                            start=False,
                            stop=(k == KT - 1),
                        )
                else:
                    xcol = xs[st // 4]
                    for kp in range(KT // 4, KT // 2):
                        nc.tensor.matmul(
                            pv[:],
                            xcol[:, 2 * kp : 2 * kp + 2, 128 * (st % 4) : 128 * (st % 4 + 1)],
                            wvt[:, 2 * kp : 2 * kp + 2, :],
                            start=False,
                            stop=(kp == KT // 2 - 1),
                            perf_mode=DR,
                        )
                nc.vector.tensor_copy(
                    vtp[st // 2].rearrange("p k (h c) -> p k h c", c=VW)[
                        :, st % 2, :, 0:HD
                    ],
                    pv.rearrange("p (h c) -> p h c", c=HD)[:],
                )
                if st < 4:
                    nc.vector.tensor_copy(
                        vt[st].rearrange("p (h c) -> p h c", c=HD + 1)[:, :, 0:HD],
                        pv.rearrange("p (h c) -> p h c", c=HD)[:],
                    )

            return [half_a, half_b]

        def emit_qk(wt, dst, m, n):
            """q^T or k^T for head-pair m, sequence superblock n (fp8 DR)."""
            state = {}

            def quarter(q):
                def _q():
                    if q == 0:
                        state["ps"] = ps_gen.tile(
                            [128, 512], F32, tag="pg", name=f"pqk{m}_{n}"
                        )
                    ps = state["ps"]
                    nc.tensor.matmul(
                        ps[:],
                        wt[:, 2 * q : 2 * q + 2, 128 * m : 128 * (m + 1)],
                        xs[n][:, 2 * q : 2 * q + 2, :],
                        start=(q == 0),
                        stop=(q == 3),
                        perf_mode=DR,
                    )
                    if q == 3:
                        nc.vector.tensor_copy(
                            dst[m][:, 512 * n : 512 * (n + 1)], ps[:]
                        )

                return _q

            return [quarter(q) for q in range(4)]

        def emit_op(m, n):
            """out^T rows [128m:128(m+1)], columns superblock n."""
            state = {}

            def half_a():
                state["ps"] = ps_gen.tile([128, 512], F32, tag="pg", name=f"pop{m}_{n}")
                for k in range(2):
                    nc.tensor.matmul(
                        state["ps"][:],
                        wot[:, k, 128 * m : 128 * (m + 1)],
                        ctxT[k][:, 512 * n : 512 * (n + 1)],
                        start=(k == 0),
                        stop=False,
                    )

            def half_b():
                ps = state["ps"]
                for k in range(2, DH // 128):
                    nc.tensor.matmul(
                        ps[:],
                        wot[:, k, 128 * m : 128 * (m + 1)],
                        ctxT[k][:, 512 * n : 512 * (n + 1)],
                        start=False,
                        stop=(k == DH // 128 - 1),
                    )
                ot = o_pool.tile([128, 512], BF16, tag="ot", name=f"ot{m}_{n}")
                nc.vector.tensor_copy(ot[:], ps[:])
                nc.sync.dma_start(
                    outT[128 * m : 128 * (m + 1), 512 * n : 512 * (n + 1)], ot[:]
                )

            return [half_a, half_b]

        # Final column-superblock out-proj, split so only a rank-128 update
        # plus an add remains after the last head finishes.
        o3_tiles = {}

        def emit_op3_partial(m):
            def unit():
                ps = ps_gen.tile([128, 512], F32, tag="pg", name=f"pop3a{m}")
                for k in range(3):
                    nc.tensor.matmul(
                        ps[:],
                        wot[:, k, 128 * m : 128 * (m + 1)],
                        ctxT[k][:, 1536:2048],
                        start=(k == 0),
                        stop=(k == 2),
                    )
                t = o3_pool.tile([128, 512], F32, tag=f"o3_{m}", name=f"o3_{m}")
                nc.vector.tensor_copy(t[:], ps[:])
                o3_tiles[m] = t

            return [unit]

        def emit_op3_final(m):
            def unit():
                ps = ps_gen.tile([128, 512], F32, tag="pg", name=f"pop3b{m}")
                nc.tensor.matmul(
                    ps[:],
                    wot[:, 3, 128 * m : 128 * (m + 1)],
                    ctxT[3][:, 1536:2048],
                    start=True,
                    stop=True,
                )
                ot = o_pool.tile([128, 512], BF16, tag="ot", name=f"ot3_{m}")
                nc.vector.tensor_tensor(
                    ot[:], o3_tiles[m][:], ps[:], mybir.AluOpType.add
                )
                nc.sync.dma_start(outT[128 * m : 128 * (m + 1), 1536:2048], ot[:])

            return [unit]

        # ---- attention ----
        pending = []  # deferred normalization closures

        def make_norm(p, I, X, cps):
            def _norm():
                cun = r_pool.tile([65, 512], F32, tag=f"cun{X}", name="cun")
                nc.vector.tensor_copy(cun[:], cps[0:65, :])
                # den row must move to partition 0: reciprocal_approx_fast is
                # custom DVE ucode and corrupts SBUF at a nonzero base
                # partition (HW-only; CoreSim doesn't model it).
                den0 = r_pool.tile([1, 512], F32, tag="den0", name="den0")
                nc.sync.dma_start(den0[0:1, :], cun[64:65, :])
                rec = r_pool.tile([1, 512], F32, tag="rec", name="rec")
                nc.vector.reciprocal_approx_fast(rec[0:1, :], den0[0:1, :])
                recb = r_pool.tile([1, 512], BF16, tag="recb", name="recb")
                nc.vector.tensor_copy(recb[0:1, :], rec[0:1, :])
                bc = ps_c.tile([65, 512], F32, tag=f"c{X}", name="bc")
                nc.tensor.matmul(
                    bc[0:64, :], onesb[0:1, 0:64], recb[0:1, :],
                    start=True, stop=True,
                )
                dst = ctxT[p][64 * X : 64 * X + 64, 512 * I : 512 * (I + 1)]
                if X == 0:
                    nc.vector.tensor_tensor(dst, cun[0:64, :], bc[0:64, :], Mult)
                else:
                    nrm = r_pool.tile([64, 512], BF16, tag="nrm", name="nrm")
                    nc.vector.tensor_tensor(nrm[:], cun[0:64, :], bc[0:64, :], Mult)
                    nc.sync.dma_start(dst, nrm[:])

            return _norm

        def run_fill(fillers, budget, urgent, rounds_left):
            while pending:
                pending.pop(0)()
            if urgent:
                k = -(-len(urgent) // max(rounds_left, 1))
                for _ in range(min(k, len(urgent))):
                    urgent.pop(0)()
            budget[0] += budget[1]
            while budget[0] >= 1.0 and fillers:
                fillers.pop(0)()
                budget[0] -= 1.0

        def attn_pair0(p, fillers, budget, urgent=None):
            """Superblock 0 (all-diagonal) for head pair p — bf16 path."""
            cps = [
                ps_c.tile([VW, 512], F32, tag=f"c{X}", name=f"cps{X}")
                for X in range(2)
            ]
            for j in range(4):
                lo = 128 * j
                sp = ps_sp.tile([128, 1024], F32, tag="sp", name="sp")
                if lo > 0:
                    # B-head's masked hole would be uninitialized PSUM under
                    # the single merged exp below.
                    nc.vector.memset(sp[:, 512 : 512 + lo], 0.0)
                for X in range(2):
                    nc.tensor.matmul(
                        sp[:, 512 * X + lo : 512 * (X + 1)],
                        kTt[p][64 * X : 64 * X + 64, 128 * j : 128 * (j + 1)],
                        qTt[p][64 * X : 64 * X + 64, lo:512],
                        start=True,
                        stop=True,
                    )
                e = e_pool.tile([128, 1024], BF16, tag="e0", bufs=4, name="e")
                nc.scalar.activation(
                    e[:, lo:1024], sp[:, lo:1024], Exp, scale=float(SCALE)
                )
                for X in range(2):
                    nc.vector.tensor_tensor(
                        e[:, 512 * X + lo : 512 * X + lo + 128],
                        e[:, 512 * X + lo : 512 * X + lo + 128],
                        trit[:],
                        Mult,
                    )
                run_fill(fillers, budget, urgent, 4 - j)
                for X in range(2):
                    nc.tensor.matmul(
                        cps[X][0:65, lo:512],
                        vt[j][:, (HD + 1) * (2 * p + X) : (HD + 1) * (2 * p + X + 1)],
                        e[:, 512 * X + lo : 512 * (X + 1)],
                        start=(j == 0),
                        stop=(j == 3),
                        skip_group_check=True,
                    )
            for X in range(2):
                pending.append(make_norm(p, 0, X, cps[X]))

        def attn_pair(p, I, fillers, budget, urgent=None):
            """Superblocks 1-3 for head pair p: fp8 DoubleRow over key-tile
            pairs.  Each rp covers key-tiles 2rp, 2rp+1 (256 keys); one DR
            A@V matmul per (rp, head)."""
            nrp = 2 * I + 2
            cps = [
                ps_c.tile([VW, 512], F32, tag=f"c{X}", name=f"cps{X}")
                for X in range(2)
            ]
            for rp in range(nrp):
                c0 = 256 if rp == 2 * I + 1 else 0
                diag = rp >= 2 * I
                # es layout: [128, X, plane, 512].  One sp tile per PLANE
                # (X-major, as in the bf16 path) keeps the one-plane-ahead
                # PSUM slack and the X-pair PE row-group overlap; the exp for
                # a plane scatters into both X slots of es, and the DR A@V
                # for head X reads its contiguous (plane, q) block.
                es = e_pool.tile([128, 2, 2, 512], F8, tag="e", name="e2")
                for pl in range(2):
                    j = 2 * rp + pl
                    sp = ps_sp.tile([128, 2, 512], F32, tag="sp", name="sp")
                    for X in range(2):
                        nc.tensor.matmul(
                            sp[:, X, c0:512],
                            kTt[p][64 * X : 64 * X + 64, 128 * j : 128 * (j + 1)],
                            qTt[p][64 * X : 64 * X + 64, 512 * I + c0 : 512 * (I + 1)],
                            start=True,
                            stop=True,
                        )
                    nc.scalar.activation(
                        es[:, :, pl, c0:512], sp[:, :, c0:512], Exp,
                        scale=float(SCALE),
                    )
                if diag:
                    # plane 0: triangle at its diag block; plane 1: zero the
                    # junk block then triangle ([zeros|tri] fp8 const).
                    for X in range(2):
                        nc.vector.tensor_tensor(
                            es[:, X, 0, c0 : c0 + 128],
                            es[:, X, 0, c0 : c0 + 128],
                            trit8[:],
                            Mult,
                        )
                        nc.vector.tensor_tensor(
                            es[:, X, 1, c0 : c0 + 256],
                            es[:, X, 1, c0 : c0 + 256],
                            zt8[:],
                            Mult,
                        )
                run_fill(fillers, budget, urgent, nrp - rp)
                for X in range(2):
                    nc.tensor.matmul(
                        cps[X][:, c0:512],
                        vtp[rp][:, :, VW * (2 * p + X) : VW * (2 * p + X + 1)],
                        es[:, X, :, c0:512],
                        start=(rp == 0),
                        stop=(rp == nrp - 1),
                        skip_group_check=True,
                        perf_mode=DR,
                    )
            for X in range(2):
                pending.append(make_norm(p, I, X, cps[X]))

        # ---- emission schedule ----
        def run_all(units):
            for u in units:
                for half in u:
                    half()

        def flat(units):
            return [half for u in units for half in u]

        # upfront: V k-tiles 0-3 and q/k for pair 0, superblock 0
        run_all([emit_v(st) for st in range(4)])
        run_all([emit_qk(wqt, qTt, 0, 0), emit_qk(wkt, kTt, 0, 0)])

        phase_fillers = [
            # during sb0: remaining sb0 q/k, V 4-7, all of sb1 q/k
            flat(
                []
                + [emit_qk(wqt, qTt, m, 0) for m in range(1, NP)]
                + [emit_qk(wkt, kTt, m, 0) for m in range(1, NP)]
                + [emit_v(st) for st in range(4, 8)]
                + [emit_qk(wqt, qTt, m, 1) for m in range(NP)]
                + [emit_qk(wkt, kTt, m, 1) for m in range(NP)]
            ),
            # during sb1: sb2 q/k
            flat(
                [emit_qk(wqt, qTt, m, 2) for m in range(NP)]
                + [emit_qk(wkt, kTt, m, 2) for m in range(NP)]
            ),
            # during sb2: sb3 q/k (V 8-11 in the urgent lane)
            flat(
                [emit_qk(wqt, qTt, m, 3) for m in range(NP)]
                + [emit_qk(wkt, kTt, m, 3) for m in range(NP)]
            ),
            # during sb3: all deferrable out-proj columns (V 12-15 in the
            # urgent lane)
            flat(
                [emit_op(m, 0) for m in range(D // 128)]
                + [emit_op(m, 1) for m in range(D // 128)]
                + [emit_op(m, 2) for m in range(D // 128)]
            ),
        ]
        # urgent lanes, popped ahead of budgeted fillers
        # (deadline-critical V generation for upcoming key-tile pairs; the
        # k=0..2 part of the final out-proj column runs during pair 3 so only
        # a rank-128 update remains after the last norm)
        urgent_lanes = {
            (2, 0): flat([emit_v(st) for st in range(8, 12)]),
            (3, 0): flat([emit_v(st) for st in range(12, 16)]),
        }

        for I in range(NB):
            fillers = phase_fillers[I]
            rounds = NP * 4 if I == 0 else NP * (2 * I + 2)
            budget = [0.999, len(fillers) / rounds]
            for p in range(NP):
                urgent = urgent_lanes.get((I, p), [])
                if I == 0:
                    attn_pair0(p, fillers, budget, urgent)
                else:
                    attn_pair(p, I, fillers, budget, urgent)
                while urgent:
                    urgent.pop(0)()
            while fillers:
                fillers.pop(0)()
        while pending:
            pending.pop(0)()
        run_all([emit_op(m, 3) for m in range(D // 128)])

    nc.compile()
    return nc


_NC_CACHE = None


def make_in_maps(x, Wq, Wk, Wv, Wo):
    bf = ml_dtypes.bfloat16
    f8 = ml_dtypes.float8_e4m3
    tri = np.triu(np.ones((128, 128), dtype=np.float32)).astype(bf)
    in_maps = []
    for c in range(NC):
        b, hg = c // 2, c % 2
        cols = slice(DH * hg, DH * (hg + 1))
        xTc = np.ascontiguousarray(np.asarray(x)[b].T)
        in_maps.append(
            {
                "xT": xTc.astype(f8),
                "xTb": np.ascontiguousarray(xTc[:, :512]).astype(bf),
                "wq": np.asarray(Wq)[:, cols].astype(f8),
                "wk": np.asarray(Wk)[:, cols].astype(f8),
                "wv": np.asarray(Wv)[:, cols].astype(f8),
                "wvb": np.asarray(Wv)[:, cols].astype(bf),
                "wo": np.asarray(Wo)[cols, :].astype(bf),
                "tri": tri,
            }
        )
    return in_maps


def kernel(x, Wq, Wk, Wv, Wo, bo):
    global _NC_CACHE
    if _NC_CACHE is None:
        _NC_CACHE = _build_nc()
    nc = _NC_CACHE

    in_maps = make_in_maps(x, Wq, Wk, Wv, Wo)
    res = run_bass_kernel_spmd(nc, in_maps, core_ids=list(range(NC)))
    out = np.empty((B, S, D), dtype=np.float32)
    bo32 = np.asarray(bo, dtype=np.float32)
    for b in range(B):
        acc = res.results[2 * b]["outT"].astype(np.float32) + res.results[2 * b + 1][
            "outT"
        ].astype(np.float32)
        out[b] = acc.T + bo32
    return out


# revision 30
# speedup vs baseline: 1.1972x; 1.1972x over previous
"""Causal multi-head attention (B=4, S=2048, D=1024, H=16, HD=64) on 8 NeuronCores.

Sharding: core c handles batch b=c//2 and head-group hg=c%2 (8 heads each).
Each core computes out^T_partial = Wo_hg^T @ ctx_hg^T for its (b, hg); the host
sums the two head-group partials per batch, transposes, and adds the bias.

v3 structure (fp8 DoubleRow generation + attention):
- Q/K projections run entirely in fp8e4m3 with DoubleRow matmuls (2 k-tiles
  per instruction, 2x PE throughput).  Score errors wash out in the softmax
  average, so fp8 q/k is accuracy-safe.
- The V projection and the attention A@V run in bf16 for key-tiles 0-3 /
  q-superblock 0: early causal rows average few keys, so fp8 noise there
  does not cancel and would blow the error budget.  Key-tiles 4-15 project
  V via fp8 DoubleRow; superblocks 1-3 run A@V as fp8 DoubleRow over
  key-tile PAIRS (one matmul per 256 keys), with exp writing fp8 directly.
- Per-head V slots are 80 wide (64 v + 1 ones + 15 pad) because the dual-fp8
  weight load requires the stationary free dim to be a multiple of 16; the
  ones column yields the softmax denominator for free at PSUM row 64.
- Diagonal-superblock masking: score matmuls extend over the masked/junk
  region (finite garbage), then e is multiplied by fp8 triangle/zero masks.
- Softmax denominators: reciprocal at partition 0 plus a K=1 broadcast
  matmul (no DMA for head A; head B normalizes at partitions 0-63 then DMAs).
- Output partials are written in bf16; the host sums in f32 and adds bias.
"""

import sys

for _p in ("/opt/trn_rl_repo",):
    if _p not in sys.path:
        sys.path.insert(0, _p)

import numpy as np
import ml_dtypes
from contextlib import ExitStack

import concourse.bacc as bacc
import concourse.tile as tile
from concourse import mybir
from concourse.bass_utils import run_bass_kernel_spmd

F32 = mybir.dt.float32
BF16 = mybir.dt.bfloat16
F8 = mybir.dt.float8e4
DR = mybir.MatmulPerfMode.DoubleRow
Exp = mybir.ActivationFunctionType.Exp
Mult = mybir.AluOpType.mult

B, S, D, H, HD = 4, 2048, 1024, 16, 64
NC = 8          # cores
HL = 8          # heads per core (head-group)
DH = HL * HD    # 512, per-core head dim
KT = D // 128   # 8 k-tiles over d_in
ST = S // 128   # 16 tiles over sequence
NB = S // 512   # 4 q-superblocks
NP = HL // 2    # 4 head pairs per core
VW = 80         # per-head V slot width: 64 v + 1 ones + 15 pad (16-aligned)
SCALE = 1.0 / np.sqrt(HD)


def _build_nc(debug=False):
    nc = bacc.Bacc("TRN2", target_bir_lowering=False)

    xT = nc.declare_dram_parameter("xT", [D, S], F8, isOutput=False)
    xTb = nc.declare_dram_parameter("xTb", [D, 512], BF16, isOutput=False)
    wq = nc.declare_dram_parameter("wq", [D, DH], F8, isOutput=False)
    wk = nc.declare_dram_parameter("wk", [D, DH], F8, isOutput=False)
    wv = nc.declare_dram_parameter("wv", [D, DH], F8, isOutput=False)
    wvb = nc.declare_dram_parameter("wvb", [D, DH], BF16, isOutput=False)
    wo = nc.declare_dram_parameter("wo", [DH, D], BF16, isOutput=False)
    tri = nc.declare_dram_parameter("tri", [128, 128], BF16, isOutput=False)
    outT = nc.declare_dram_parameter("outT", [D, S], BF16, isOutput=True)

    with tile.TileContext(nc) as tc, ExitStack() as ctx:
        const_pool = ctx.enter_context(tc.tile_pool(name="const", bufs=1))
        x_pool = ctx.enter_context(tc.tile_pool(name="x", bufs=1))
        w_pool = ctx.enter_context(tc.tile_pool(name="w", bufs=1))
        qk_pool = ctx.enter_context(tc.tile_pool(name="qk", bufs=1))
        v_pool = ctx.enter_context(tc.tile_pool(name="v", bufs=1))
        ctxT_pool = ctx.enter_context(tc.tile_pool(name="ctxT", bufs=1))
        e_pool = ctx.enter_context(tc.tile_pool(name="e", bufs=8))
        r_pool = ctx.enter_context(tc.tile_pool(name="r", bufs=1))
        o_pool = ctx.enter_context(tc.tile_pool(name="o", bufs=2))
        o3_pool = ctx.enter_context(tc.tile_pool(name="o3", bufs=1))
        ps_sp = ctx.enter_context(tc.tile_pool(name="ps_sp", bufs=2, space="PSUM"))
        ps_c = ctx.enter_context(tc.tile_pool(name="ps_c", bufs=1, space="PSUM"))
        ps_gen = ctx.enter_context(tc.tile_pool(name="ps_gen", bufs=2, space="PSUM"))

        # ---- constants ----
        trit = const_pool.tile([128, 128], BF16)
        nc.sync.dma_start(trit[:], tri[:])
        onesb = const_pool.tile([1, 64], BF16)
        nc.vector.memset(onesb[:], 1.0)
        # fp8 masks for the DoubleRow diag rounds: triangle, and [zeros|triangle]
        trit8 = const_pool.tile([128, 128], F8)
        nc.vector.tensor_copy(trit8[:], trit[:])
        zt8 = const_pool.tile([128, 256], F8)
        nc.vector.memset(zt8[:, 0:128], 0.0)
        nc.vector.tensor_copy(zt8[:, 128:256], trit[:])

        # ---- inputs ----
        # Per-k 2D slices (multi-dim dram APs under-synchronize on HW).
        # Priority order on one queue: a DMA's descriptors go out before the
        # next trigger's, so the front-critical tensors (bf16 wv + bf16 x
        # superblock 0 for emit_v(0..3), then q/k weights) finish first.
        wvt = w_pool.tile([128, KT, DH], F8, name="wvt")
        wvbt = w_pool.tile([128, KT, DH], BF16, name="wvbt")
        wqt = w_pool.tile([128, KT, DH], F8, name="wqt")
        wkt = w_pool.tile([128, KT, DH], F8, name="wkt")
        wot = w_pool.tile([128, DH // 128, D], BF16, name="wot")
        xs = [x_pool.tile([128, KT, 512], F8, name=f"xs{_}") for _ in range(NB)]
        xs0b = x_pool.tile([128, KT, 512], BF16, name="xs0b")

        def xsrc(s):
            return xT[:, 512 * s : 512 * (s + 1)].rearrange("(k p) c -> p k c", p=128)

        for k in range(KT):
            nc.sync.dma_start(wvbt[:, k, :], wvb[128 * k : 128 * (k + 1), :])
            nc.sync.dma_start(xs0b[:, k, :], xTb[128 * k : 128 * (k + 1), :])
        nc.sync.dma_start(wqt[:], wq.rearrange("(k p) c -> p k c", p=128)[:])
        nc.sync.dma_start(wkt[:], wk.rearrange("(k p) c -> p k c", p=128)[:])
        nc.sync.dma_start(xs[0][:], xsrc(0)[:])
        nc.sync.dma_start(xs[1][:], xsrc(1)[:])
        nc.sync.dma_start(xs[2][:], xsrc(2)[:])
        nc.sync.dma_start(wvt[:], wv.rearrange("(k p) c -> p k c", p=128)[:])
        nc.sync.dma_start(xs[3][:], xsrc(3)[:])
        nc.sync.dma_start(wot[:], wo.rearrange("(k p) c -> p k c", p=128)[:])

        # ---- persistent activations ----
        qTt = [qk_pool.tile([128, S], BF16, name=f"qT{_}") for _ in range(NP)]
        kTt = [qk_pool.tile([128, S], BF16, name=f"kT{_}") for _ in range(NP)]
        # bf16 V (65-wide slots) for key-tiles 0-3: superblock-0 A@V path
        vt = [v_pool.tile([128, HL * (HD + 1)], BF16, name=f"v{_}") for _ in range(4)]
        # fp8 V key-tile pairs (80-wide slots) for the DoubleRow A@V path
        vtp = [v_pool.tile([128, 2, HL * VW], F8, name=f"vp{_}") for _ in range(ST // 2)]
        ctxT = [ctxT_pool.tile([128, S], BF16, name=f"ctxT{_}") for _ in range(NP)]

        # ---- generation units ----
        # Each unit is one PSUM accumulation chain, split into two half-unit
        # closures for fine-grained interleaving with the attention rounds.
        def emit_v(st):
            """V for k-tile st.

            st 0-3 run in bf16 (both vt and vtp copies are written): early
            causal rows attend over few keys, so fp8 noise there does not
            average out.  st 4-15 use the fp8 DoubleRow path."""
            state = {}

            def half_a():
                nc.vector.memset(
                    vtp[st // 2].rearrange("p k (h c) -> p k h c", c=VW)[
                        :, st % 2, :, HD : VW
                    ],
                    1.0,
                )
                if st < 4:
                    nc.vector.memset(
                        vt[st].rearrange("p (h c) -> p h c", c=HD + 1)[:, :, HD], 1.0
                    )
                state["pv"] = ps_gen.tile([128, 512], F32, tag="pg", name=f"pv{st}")
                if st < 4:
                    for k in range(KT // 2):
                        nc.tensor.matmul(
                            state["pv"][:],
                            xs0b[:, k, 128 * st : 128 * (st + 1)],
                            wvbt[:, k, :],
                            start=(k == 0),
                            stop=False,
                        )
                else:
                    xcol = xs[st // 4]
                    for kp in range(KT // 4):
                        nc.tensor.matmul(
                            state["pv"][:],
                            xcol[:, 2 * kp : 2 * kp + 2, 128 * (st % 4) : 128 * (st % 4 + 1)],
                            wvt[:, 2 * kp : 2 * kp + 2, :],
                            start=(kp == 0),
                            stop=False,
                            perf_mode=DR,
                        )

            def half_b():
                pv = state["pv"]
                if st < 4:
                    for k in range(KT // 2, KT):
                        nc.tensor.matmul(
                            pv[:],
                            xs0b[:, k, 128 * st : 128 * (st + 1)],
                            wvbt[:, k, :],
                            start=False,
                            stop=(k == KT - 1),
                        )
                else:
                    xcol = xs[st // 4]
                    for kp in range(KT // 4, KT // 2):
                        nc.tensor.matmul(
                            pv[:],
                            xcol[:, 2 * kp : 2 * kp + 2, 128 * (st % 4) : 128 * (st % 4 + 1)],
                            wvt[:, 2 * kp : 2 * kp + 2, :],
                            start=False,
                            stop=(kp == KT // 2 - 1),
                            perf_mode=DR,
                        )
                nc.vector.tensor_copy(
                    vtp[st // 2].rearrange("p k (h c) -> p k h c", c=VW)[
                        :, st % 2, :, 0:HD
                    ],
                    pv.rearrange("p (h c) -> p h c", c=HD)[:],
                )
                if st < 4:
                    nc.vector.tensor_copy(
                        vt[st].rearrange("p (h c) -> p h c", c=HD + 1)[:, :, 0:HD],
                        pv.rearrange("p (h c) -> p h c", c=HD)[:],
                    )

            return [half_a, half_b]

        def emit_qk(wt, dst, m, n):
            """q^T or k^T for head-pair m, sequence superblock n (fp8 DR)."""
            state = {}

            def quarter(q):
                def _q():
                    if q == 0:
                        state["ps"] = ps_gen.tile(
                            [128, 512], F32, tag="pg", name=f"pqk{m}_{n}"
                        )
                    ps = state["ps"]
                    nc.tensor.matmul(
                        ps[:],
                        wt[:, 2 * q : 2 * q + 2, 128 * m : 128 * (m + 1)],
                        xs[n][:, 2 * q : 2 * q + 2, :],
                        start=(q == 0),
                        stop=(q == 3),
                        perf_mode=DR,
                    )
                    if q == 3:
                        nc.vector.tensor_copy(
                            dst[m][:, 512 * n : 512 * (n + 1)], ps[:]
                        )

                return _q

            return [quarter(q) for q in range(4)]

        def emit_op(m, n):
            """out^T rows [128m:128(m+1)], columns superblock n."""
            state = {}

            def half_a():
                state["ps"] = ps_gen.tile([128, 512], F32, tag="pg", name=f"pop{m}_{n}")
                for k in range(2):
                    nc.tensor.matmul(
                        state["ps"][:],
                        wot[:, k, 128 * m : 128 * (m + 1)],
                        ctxT[k][:, 512 * n : 512 * (n + 1)],
                        start=(k == 0),
                        stop=False,
                    )

            def half_b():
                ps = state["ps"]
                for k in range(2, DH // 128):
                    nc.tensor.matmul(
                        ps[:],
                        wot[:, k, 128 * m : 128 * (m + 1)],
                        ctxT[k][:, 512 * n : 512 * (n + 1)],
                        start=False,
                        stop=(k == DH // 128 - 1),
                    )
                ot = o_pool.tile([128, 512], BF16, tag="ot", name=f"ot{m}_{n}")
                nc.vector.tensor_copy(ot[:], ps[:])
                nc.sync.dma_start(
                    outT[128 * m : 128 * (m + 1), 512 * n : 512 * (n + 1)], ot[:]
                )

            return [half_a, half_b]

        # Final column-superblock out-proj, split so only a rank-128 update
        # plus an add remains after the last head finishes.
        o3_tiles = {}

        def emit_op3_partial(m):
            def unit():
                ps = ps_gen.tile([128, 512], F32, tag="pg", name=f"pop3a{m}")
                for k in range(3):
                    nc.tensor.matmul(
                        ps[:],
                        wot[:, k, 128 * m : 128 * (m + 1)],
                        ctxT[k][:, 1536:2048],
                        start=(k == 0),
                        stop=(k == 2),
                    )
                t = o3_pool.tile([128, 512], F32, tag=f"o3_{m}", name=f"o3_{m}")
                nc.vector.tensor_copy(t[:], ps[:])
                o3_tiles[m] = t

            return [unit]

        def emit_op3_final(m):
            def unit():
                ps = ps_gen.tile([128, 512], F32, tag="pg", name=f"pop3b{m}")
                nc.tensor.matmul(
                    ps[:],
                    wot[:, 3, 128 * m : 128 * (m + 1)],
                    ctxT[3][:, 1536:2048],
                    start=True,
                    stop=True,
                )
                ot = o_pool.tile([128, 512], BF16, tag="ot", name=f"ot3_{m}")
                nc.vector.tensor_tensor(
                    ot[:], o3_tiles[m][:], ps[:], mybir.AluOpType.add
                )
                nc.sync.dma_start(outT[128 * m : 128 * (m + 1), 1536:2048], ot[:])

            return [unit]

        # ---- attention ----
        pending = []  # deferred normalization closures

        def make_norm(p, I, X, cps):
            def _norm():
                cun = r_pool.tile([65, 512], F32, tag=f"cun{X}", name="cun")
                nc.vector.tensor_copy(cun[:], cps[0:65, :])
                # den row must move to partition 0: reciprocal_approx_fast is
                # custom DVE ucode and corrupts SBUF at a nonzero base
                # partition (HW-only; CoreSim doesn't model it).
                den0 = r_pool.tile([1, 512], F32, tag="den0", name="den0")
                nc.sync.dma_start(den0[0:1, :], cun[64:65, :])
                rec = r_pool.tile([1, 512], F32, tag="rec", name="rec")
                nc.vector.reciprocal_approx_fast(rec[0:1, :], den0[0:1, :])
                recb = r_pool.tile([1, 512], BF16, tag="recb", name="recb")
                nc.vector.tensor_copy(recb[0:1, :], rec[0:1, :])
                bc = ps_c.tile([65, 512], F32, tag=f"c{X}", name="bc")
                nc.tensor.matmul(
                    bc[0:64, :], onesb[0:1, 0:64], recb[0:1, :],
                    start=True, stop=True,
                )
                dst = ctxT[p][64 * X : 64 * X + 64, 512 * I : 512 * (I + 1)]
                if X == 0:
                    nc.vector.tensor_tensor(dst, cun[0:64, :], bc[0:64, :], Mult)
                else:
                    nrm = r_pool.tile([64, 512], BF16, tag="nrm", name="nrm")
                    nc.vector.tensor_tensor(nrm[:], cun[0:64, :], bc[0:64, :], Mult)
                    nc.sync.dma_start(dst, nrm[:])

            return _norm

        def run_fill(fillers, budget, urgent, rounds_left):
            while pending:
                pending.pop(0)()
            if urgent:
                k = -(-len(urgent) // max(rounds_left, 1))
                for _ in range(min(k, len(urgent))):
                    urgent.pop(0)()
            budget[0] += budget[1]
            while budget[0] >= 1.0 and fillers:
                fillers.pop(0)()
                budget[0] -= 1.0

        def attn_pair0(p, fillers, budget, urgent=None):
            """Superblock 0 (all-diagonal) for head pair p — bf16 path."""
            cps = [
                ps_c.tile([VW, 512], F32, tag=f"c{X}", name=f"cps{X}")
                for X in range(2)
            ]
            for j in range(4):
                lo = 128 * j
                sp = ps_sp.tile([128, 1024], F32, tag="sp", name="sp")
                if lo > 0:
                    # B-head's masked hole would be uninitialized PSUM under
                    # the single merged exp below.
                    nc.vector.memset(sp[:, 512 : 512 + lo], 0.0)
                for X in range(2):
                    nc.tensor.matmul(
                        sp[:, 512 * X + lo : 512 * (X + 1)],
                        kTt[p][64 * X : 64 * X + 64, 128 * j : 128 * (j + 1)],
                        qTt[p][64 * X : 64 * X + 64, lo:512],
                        start=True,
                        stop=True,
                    )
                e = e_pool.tile([128, 1024], BF16, tag="e0", bufs=4, name="e")
                nc.scalar.activation(
                    e[:, lo:1024], sp[:, lo:1024], Exp, scale=float(SCALE)
                )
                for X in range(2):
                    nc.vector.tensor_tensor(
                        e[:, 512 * X + lo : 512 * X + lo + 128],
                        e[:, 512 * X + lo : 512 * X + lo + 128],
                        trit[:],
                        Mult,
                    )
                run_fill(fillers, budget, urgent, 4 - j)
                for X in range(2):
                    nc.tensor.matmul(
                        cps[X][0:65, lo:512],
                        vt[j][:, (HD + 1) * (2 * p + X) : (HD + 1) * (2 * p + X + 1)],
                        e[:, 512 * X + lo : 512 * (X + 1)],
                        start=(j == 0),
                        stop=(j == 3),
                        skip_group_check=True,
                    )
            for X in range(2):
                pending.append(make_norm(p, 0, X, cps[X]))

        def attn_pair(p, I, fillers, budget, urgent=None):
            """Superblocks 1-3 for head pair p: fp8 DoubleRow over key-tile
            pairs.  Each rp covers key-tiles 2rp, 2rp+1 (256 keys); one DR
            A@V matmul per (rp, head)."""
            nrp = 2 * I + 2
            cps = [
                ps_c.tile([VW, 512], F32, tag=f"c{X}", name=f"cps{X}")
                for X in range(2)
            ]
            deferred = [None]

            def flush_av():
                if deferred[0] is None:
                    return
                frp, fc0, fes = deferred[0]
                for X in range(2):
                    nc.tensor.matmul(
                        cps[X][:, fc0:512],
                        vtp[frp][:, :, VW * (2 * p + X) : VW * (2 * p + X + 1)],
                        fes[:, X, :, fc0:512],
                        start=(frp == 0),
                        stop=(frp == nrp - 1),
                        skip_group_check=True,
                        perf_mode=DR,
                    )
                deferred[0] = None

            for rp in range(nrp):
                c0 = 256 if rp == 2 * I + 1 else 0
                diag = rp >= 2 * I
                # es layout: [128, X, plane, 512].  One sp tile per PLANE
                # (X-major, as in the bf16 path) keeps the one-plane-ahead
                # PSUM slack and the X-pair PE row-group overlap; the exp for
                # a plane scatters into both X slots of es, and the DR A@V
                # for head X reads its contiguous (plane, q) block.
                es = e_pool.tile([128, 2, 2, 512], F8, tag="e", name="e2")
                for pl in range(2):
                    j = 2 * rp + pl
                    sp = ps_sp.tile([128, 2, 512], F32, tag="sp", name="sp")
                    for X in range(2):
                        nc.tensor.matmul(
                            sp[:, X, c0:512],
                            kTt[p][64 * X : 64 * X + 64, 128 * j : 128 * (j + 1)],
                            qTt[p][64 * X : 64 * X + 64, 512 * I + c0 : 512 * (I + 1)],
                            start=True,
                            stop=True,
                        )
                    nc.scalar.activation(
                        es[:, :, pl, c0:512], sp[:, :, c0:512], Exp,
                        scale=float(SCALE),
                    )
                flush_av()
                if diag:
                    # plane 0: triangle at its diag block; plane 1: zero the
                    # junk block then triangle ([zeros|tri] fp8 const).
                    for X in range(2):
                        nc.vector.tensor_tensor(
                            es[:, X, 0, c0 : c0 + 128],
                            es[:, X, 0, c0 : c0 + 128],
                            trit8[:],
                            Mult,
                        )
                        nc.vector.tensor_tensor(
                            es[:, X, 1, c0 : c0 + 256],
                            es[:, X, 1, c0 : c0 + 256],
                            zt8[:],
                            Mult,
                        )
                run_fill(fillers, budget, urgent, nrp - rp)
                deferred[0] = (rp, c0, es)
            flush_av()
            for X in range(2):
                pending.append(make_norm(p, I, X, cps[X]))

        # ---- emission schedule ----
        def run_all(units):
            for u in units:
                for half in u:
                    half()

        def flat(units):
            return [half for u in units for half in u]

        # upfront: V k-tiles 0-3 and q/k for pair 0, superblock 0
        run_all([emit_v(st) for st in range(4)])
        run_all([emit_qk(wqt, qTt, 0, 0), emit_qk(wkt, kTt, 0, 0)])

        phase_fillers = [
            # during sb0: remaining sb0 q/k, V 4-7, all of sb1 q/k
            flat(
                []
                + [emit_qk(wqt, qTt, m, 0) for m in range(1, NP)]
                + [emit_qk(wkt, kTt, m, 0) for m in range(1, NP)]
                + [emit_v(st) for st in range(4, 8)]
                + [emit_qk(wqt, qTt, m, 1) for m in range(NP)]
                + [emit_qk(wkt, kTt, m, 1) for m in range(NP)]
            ),
            # during sb1: sb2 q/k
            flat(
                [emit_qk(wqt, qTt, m, 2) for m in range(NP)]
                + [emit_qk(wkt, kTt, m, 2) for m in range(NP)]
            ),
            # during sb2: sb3 q/k (V 8-11 in the urgent lane)
            flat(
                [emit_qk(wqt, qTt, m, 3) for m in range(NP)]
                + [emit_qk(wkt, kTt, m, 3) for m in range(NP)]
            ),
            # during sb3: all deferrable out-proj columns (V 12-15 in the
            # urgent lane)
            flat(
                [emit_op(m, 0) for m in range(D // 128)]
                + [emit_op(m, 1) for m in range(D // 128)]
                + [emit_op(m, 2) for m in range(D // 128)]
            ),
        ]
        # urgent lanes, popped ahead of budgeted fillers
        # (deadline-critical V generation for upcoming key-tile pairs; the
        # k=0..2 part of the final out-proj column runs during pair 3 so only
        # a rank-128 update remains after the last norm)
        urgent_lanes = {
            (2, 0): flat([emit_v(st) for st in range(8, 12)]),
            (3, 0): flat([emit_v(st) for st in range(12, 16)]),
        }

        for I in range(NB):
            fillers = phase_fillers[I]
            rounds = NP * 4 if I == 0 else NP * (2 * I + 2)
            budget = [0.999, len(fillers) / rounds]
            for p in range(NP):
                urgent = urgent_lanes.get((I, p), [])
                if I == 0:
                    attn_pair0(p, fillers, budget, urgent)
                else:
                    attn_pair(p, I, fillers, budget, urgent)
                while urgent:
                    urgent.pop(0)()
            while fillers:
                fillers.pop(0)()
        while pending:
            pending.pop(0)()
        run_all([emit_op(m, 3) for m in range(D // 128)])

    nc.compile()
    return nc


_NC_CACHE = None


def make_in_maps(x, Wq, Wk, Wv, Wo):
    bf = ml_dtypes.bfloat16
    f8 = ml_dtypes.float8_e4m3
    tri = np.triu(np.ones((128, 128), dtype=np.float32)).astype(bf)
    in_maps = []
    for c in range(NC):
        b, hg = c // 2, c % 2
        cols = slice(DH * hg, DH * (hg + 1))
        xTc = np.ascontiguousarray(np.asarray(x)[b].T)
        in_maps.append(
            {
                "xT": xTc.astype(f8),
                "xTb": np.ascontiguousarray(xTc[:, :512]).astype(bf),
                "wq": np.asarray(Wq)[:, cols].astype(f8),
                "wk": np.asarray(Wk)[:, cols].astype(f8),
                "wv": np.asarray(Wv)[:, cols].astype(f8),
                "wvb": np.asarray(Wv)[:, cols].astype(bf),
                "wo": np.asarray(Wo)[cols, :].astype(bf),
                "tri": tri,
            }
        )
    return in_maps


def kernel(x, Wq, Wk, Wv, Wo, bo):
    global _NC_CACHE
    if _NC_CACHE is None:
        _NC_CACHE = _build_nc()
    nc = _NC_CACHE

    in_maps = make_in_maps(x, Wq, Wk, Wv, Wo)
    res = run_bass_kernel_spmd(nc, in_maps, core_ids=list(range(NC)))
    out = np.empty((B, S, D), dtype=np.float32)
    bo32 = np.asarray(bo, dtype=np.float32)
    for b in range(B):
        acc = res.results[2 * b]["outT"].astype(np.float32) + res.results[2 * b + 1][
            "outT"
        ].astype(np.float32)
        out[b] = acc.T + bo32
    return out
